# revision 2
# baseline (speedup 1.0000x reference)
"""Gemma3 sliding-window attention on 8 Trainium2 NeuronCores — bf16 rework.

Sharding: core c handles batch b=c//4 and head-group g=c%4 (4 of 16 q heads,
2 of 8 kv heads). wq/wk/wv column-split, wo row-split; the 4 partial outputs
per batch are summed on host.

All matmul inputs are bf16 (host-converted); PSUM accumulates f32.
Q/K produced transposed (d on partitions), scores computed transposed [k,q]
so PV needs no transposes; softmax normalisation deferred (flash-style).
Denominator = DVE-summed probs + one ones-matmul per (head, chunk).
RMS-norm partition reduction runs on the Pool engine (partition_all_reduce).
Output projection interleaved with attention per q-chunk; y DMAed straight
from PSUM on the Pool DGE queue.
"""

import math
import numpy as np

import concourse.bacc as bacc
import concourse.mybir as mybir
import concourse.tile as tile
import concourse.bass_isa as bass_isa
from concourse.bass_utils import run_bass_kernel_spmd

dt = mybir.dt
AFT = mybir.ActivationFunctionType

B, S, H = 2, 2048, 2048
NQ, NKV, D = 16, 8, 128          # global heads
NQC, NKVC = 4, 2                 # heads per core
WIN = 1024
EPS = 1e-6
THETA = 10000.0
P = 128
SCP = 256                        # seq chunk
NHT = H // P                     # 16 hidden tiles
NST = S // P                     # 16 seq tiles
NCH = S // SCP                   # 8 chunks
WT = WIN // P                    # 8 window tiles

_CACHE = {}
import os
PHASES = int(os.environ.get("KERNEL_PHASES", "3"))


def _attn_blocks(qc):
    """[(t, q_lo, q_hi, mask)] for q-chunk qc; mask in (None,'d','e') per sub."""
    u0 = 2 * qc
    out = []
    for t in range(max(0, u0 - WT), u0 + 2):
        if t == u0 + 1:
            lo, hi = P, 2 * P
        elif t == u0 - WT:
            lo, hi = 0, P
        else:
            lo, hi = 0, 2 * P
        masks = []
        for sub in (0, 1):
            s_lo, s_hi = sub * P, (sub + 1) * P
            if s_lo < lo or s_hi > hi:
                continue
            dd = (u0 + sub) - t
            if dd == 0:
                masks.append((s_lo, "d"))
            elif dd == WT:
                masks.append((s_lo, "e"))
        out.append((t, lo, hi, masks))
    return out


def _build_nc():
    if "nc" in _CACHE:
        return _CACHE["nc"]
    nc = bacc.Bacc("TRN2", target_bir_lowering=False, debug=False, num_devices=8)
    f32, bf16 = dt.float32, dt.bfloat16

    hsT = nc.dram_tensor("hsT", [H, S], bf16, kind="ExternalInput").ap()
    wqT = nc.dram_tensor("wqT", [H, NQC * D], bf16, kind="ExternalInput").ap()
    wkT = nc.dram_tensor("wkT", [H, NKVC * D], bf16, kind="ExternalInput").ap()
    wvT = nc.dram_tensor("wvT", [H, NKVC * D], bf16, kind="ExternalInput").ap()
    woT = nc.dram_tensor("woT", [NQC * D, H], bf16, kind="ExternalInput").ap()
    tabcat = nc.dram_tensor("tabcat", [4 * D, S], bf16, kind="ExternalInput").ap()
    rqT = nc.dram_tensor("rqT", [D, D], bf16, kind="ExternalInput").ap()
    rkT = nc.dram_tensor("rkT", [D, D], bf16, kind="ExternalInput").ap()
    onesd = nc.dram_tensor("onesd", [P, P], bf16, kind="ExternalInput").ap()
    dmask = nc.dram_tensor("dmask", [P, P], bf16, kind="ExternalInput").ap()
    emask = nc.dram_tensor("emask", [P, P], bf16, kind="ExternalInput").ap()
    yT = nc.dram_tensor("yT", [H, S], bf16, kind="ExternalOutput").ap()

    with tile.TileContext(nc) as tc:
        with (
            tc.tile_pool(name="const", bufs=1) as cpool,
            tc.tile_pool(name="qkv", bufs=1) as qkv,
            tc.tile_pool(name="wts", bufs=1) as wts,
        ):
            ones_sb = cpool.tile([P, P], bf16, tag="ones")
            dm_sb = cpool.tile([P, P], bf16, tag="dm")
            em_sb = cpool.tile([P, P], bf16, tag="em")
            rq_sb = cpool.tile([D, D], bf16, tag="rq")
            rk_sb = cpool.tile([D, D], bf16, tag="rk")
            eps_sb = cpool.tile([P, 1], f32, tag="eps")
            nc.vector.memset(eps_sb[:], EPS)
            warm_sb = cpool.tile([P, P], bf16, tag="warm")
            nc.vector.memset(warm_sb[:], 0.0)

            wq_sb = wts.tile([P, NHT, NQC * D], bf16, tag="wq")
            wk_sb = wts.tile([P, NHT, NKVC * D], bf16, tag="wk")
            wv_sb = wts.tile([P, NHT, NKVC * D], bf16, tag="wv")
            wo_sb = wts.tile([P, NQC, H], bf16, tag="wo")


            qn_sb = qkv.tile([P, NQC, S], bf16, tag="qn")
            kn_sb = qkv.tile([P, NKVC, S], bf16, tag="kn")
            v_sb = qkv.tile([P, NST, NKVC * D], bf16, tag="v")
            attn_sb = qkv.tile([P, NQC, S], bf16, tag="attn")

            # ---------------- phase 1: QKV projections + RMSNorm + RoPE ----
            with (
                tc.tile_pool(name="hsp", bufs=2) as hsp,
                tc.tile_pool(name="tabp", bufs=2) as tabp,
                tc.tile_pool(name="tmp1", bufs=3) as tmp1,
                tc.tile_pool(name="xnp", bufs=3) as xnp,
                tc.tile_pool(name="ropep", bufs=4) as ropep,
                tc.tile_pool(name="psq", bufs=3, space="PSUM") as psq,
                tc.tile_pool(name="pskv", bufs=2, space="PSUM") as pskv,
                tc.tile_pool(name="psv", bufs=1, space="PSUM") as psv,
                tc.tile_pool(name="psrb", bufs=2, space="PSUM") as psrb,
            ):
                # PE warmup: keep the tensor engine busy (and its p-state
                # ramping) while the first input DMAs stream in
                warm_ps = psv.tile([P, 2, NKVC * D], f32, tag="vproj", name="warmps")
                for _ in range(32):
                    nc.tensor.matmul(warm_ps[:, 0, 0:P], warm_sb[:], warm_sb[:],
                                     start=True, stop=True)

                import collections as _c
                rope_q = _c.deque()

                def emit_rope(kind, pr, xn, s0, tabs):
                    rot_sb = rq_sb if kind == "q" else rk_sb
                    cos_t = tabs["cosq" if kind == "q" else "cosk"]
                    sin_t = tabs["sinq" if kind == "q" else "sink"]
                    dst = qn_sb if kind == "q" else kn_sb
                    rb = psrb.tile([P, 2, SCP], f32, tag="rb")
                    nc.tensor.matmul(rb[:, :, :], rot_sb[:], xn[:, :, :],
                                     start=True, stop=True)
                    for j in range(2):
                        m = 2 * pr + j
                        tc_ = ropep.tile([P, SCP], bf16, tag="tc")
                        nc.vector.tensor_mul(tc_[:], xn[:, j, :], cos_t)
                        ts_ = ropep.tile([P, SCP], bf16, tag="ts")
                        nc.vector.tensor_mul(ts_[:], rb[:, j, :], sin_t)
                        nc.vector.tensor_add(
                            dst[:, m, s0:s0 + SCP], tc_[:], ts_[:])

                for sc in range(NCH):
                    s0 = sc * SCP
                    hs_sb = hsp.tile([P, NHT, SCP], bf16, tag="hs")
                    if sc == 0:
                        # interleave hs and wq (heads 0-1) in 4-ht pieces so
                        # the q0 section starts as soon as the first piece lands
                        for hp in range(4):
                            h0, h1 = hp * 4, (hp + 1) * 4
                            nc.sync.dma_start(
                                out=hs_sb[:, h0:h1, :],
                                in_=hsT[h0 * P:h1 * P, s0:s0 + SCP]
                                    .rearrange("(ht p) s -> p ht s", p=P))
                            nc.sync.dma_start(
                                out=wq_sb[:, h0:h1, 0:2 * D],
                                in_=wqT[h0 * P:h1 * P, 0:2 * D]
                                    .rearrange("(ht p) d -> p ht d", p=P))
                    else:
                        nc.sync.dma_start(
                            out=hs_sb[:, :, :],
                            in_=hsT[:, s0:s0 + SCP].rearrange("(ht p) s -> p ht s", p=P))
                    tt = tabp.tile([D, 4, SCP], bf16, tag="tabs")
                    if sc == 0:
                        # just-in-time weight order: q1's wq half, then wk,
                        # then rope tables/consts, then wv (v-proj is last)
                        nc.scalar.dma_start(
                            out=wq_sb[:, :, 2 * D:4 * D],
                            in_=wqT[:, 2 * D:4 * D].rearrange("(ht p) d -> p ht d", p=P))
                        nc.scalar.dma_start(
                            out=wk_sb[:, :, :],
                            in_=wkT[:, :].rearrange("(ht p) d -> p ht d", p=P))
                    nc.scalar.dma_start(
                        out=tt[:, :, :],
                        in_=tabcat[:, s0:s0 + SCP].rearrange("(tb p) s -> p tb s", p=D))
                    if sc == 0:
                        nc.sync.dma_start(out=rq_sb[:], in_=rqT[:])
                        nc.sync.dma_start(out=rk_sb[:], in_=rkT[:])
                        nc.scalar.dma_start(
                            out=wv_sb[:, :, :],
                            in_=wvT[:, :].rearrange("(ht p) d -> p ht d", p=P))
                        nc.scalar.dma_start(out=ones_sb[:], in_=onesd[:])
                        nc.scalar.dma_start(out=dm_sb[:], in_=dmask[:])
                        nc.scalar.dma_start(out=em_sb[:], in_=emask[:])
                    tabs = {nm: tt[:, i, :] for i, nm in enumerate(("cosq", "sinq", "cosk", "sink"))}

                    for kind, pr in (("q", 0), ("q", 1), ("k", 0)):
                        w_sb = wq_sb if kind == "q" else wk_sb
                        pool = psq if kind == "q" else pskv
                        pp = pool.tile([P, 2, SCP], f32, tag="proj")
                        for ht in range(NHT):
                            for j in range(2):
                                m = 2 * pr + j
                                nc.tensor.matmul(
                                    pp[:, j, :], w_sb[:, ht, m * D:(m + 1) * D],
                                    hs_sb[:, ht, :],
                                    start=(ht == 0 and j == 0),
                                    stop=(ht == NHT - 1 and j == 1),
                                    skip_group_check=True)
                        if len(rope_q) >= 2:
                            emit_rope(*rope_q.popleft())
                        sq = tmp1.tile([P, 2, SCP], f32, tag="sq")
                        nc.scalar.square(sq[:, :, :], pp[:, :, :])
                        ms = tmp1.tile([P, 2, SCP], f32, tag="ms")
                        nc.gpsimd.partition_all_reduce(ms[:, :, :], sq[:, :, :], P,
                                                       bass_isa.ReduceOp.add)
                        sd = tmp1.tile([P, 2, SCP], f32, tag="sd")
                        nc.scalar.activation(sd[:, :, :], ms[:, :, :], AFT.Sqrt,
                                             bias=eps_sb[:], scale=1.0 / D)
                        inv = tmp1.tile([P, 2, SCP], f32, tag="inv")
                        nc.vector.reciprocal(inv[:, :, :], sd[:, :, :])
                        xn = xnp.tile([P, 2, SCP], bf16, tag="xn")
                        nc.vector.tensor_mul(xn[:, :, :], pp[:, :, :], inv[:, :, :])
                        rope_q.append((kind, pr, xn, s0, tabs))

                    # v: natural layout
                    vp = psv.tile([P, 2, NKVC * D], f32, tag="vproj")
                    for ss in range(2):
                        for ht in range(NHT):
                            nc.tensor.matmul(
                                vp[:, ss, :], hs_sb[:, ht, ss * P:(ss + 1) * P],
                                wv_sb[:, ht, :],
                                start=(ss == 0 and ht == 0),
                                stop=(ss == 1 and ht == NHT - 1),
                                skip_group_check=True)
                    if len(rope_q) >= 2:
                        emit_rope(*rope_q.popleft())
                    st = sc * 2
                    nc.vector.tensor_copy(v_sb[:, st:st + 2, :], vp[:, :, :])
                for item in list(rope_q):
                    emit_rope(*rope_q.popleft())

            # ---------------- phase 2+3: attention + output projection -----
            with (
                tc.tile_pool(name="probs", bufs=8) as probs,
                tc.tile_pool(name="pssum", bufs=4) as pssum,
                tc.tile_pool(name="invp", bufs=4) as invp,
                tc.tile_pool(name="ysb", bufs=3) as ysbp,
                tc.tile_pool(name="slab", bufs=3, space="PSUM") as slab,
                tc.tile_pool(name="psad", bufs=3, space="PSUM") as psad,
                tc.tile_pool(name="psy", bufs=2, space="PSUM") as psy,
            ):
                nc.sync.dma_start(
                    out=wo_sb[:, :, :],
                    in_=woT[:, :].rearrange("(dto p) h -> p dto h", p=P))
                def emit_scores(h, qc):
                    """scores+exp for (h, qc): returns [(p_sb, off_in_slab, t, lo, hi, masks)]"""
                    kvh = h // 2
                    blocks = _attn_blocks(qc)
                    out = []
                    cur = None  # (slab_tile, used)
                    packed = []  # (tile, width, [(off, t, lo, hi, masks)])
                    for (t, lo, hi, masks) in blocks:
                        w = hi - lo
                        if cur is None or cur[1] + w > 512:
                            cur = [slab.tile([P, 512], f32, tag="slab", name="slabt"), 0, []]
                            packed.append(cur)
                        off = cur[1]
                        nc.tensor.matmul(
                            cur[0][:, off:off + w],
                            kn_sb[:, kvh, t * P:(t + 1) * P],
                            qn_sb[:, h, qc * SCP + lo:qc * SCP + hi],
                            start=(off == 0), stop=False,
                            skip_group_check=True)
                        cur[2].append((off, t, lo, hi, masks))
                        cur[1] += w
                    for (stile, used, blks) in packed:
                        p_sb = probs.tile([P, 512], bf16, tag="p")
                        nc.scalar.activation(p_sb[:, 0:used], stile[:, 0:used], AFT.Exp)
                        for (off, t, lo, hi, masks) in blks:
                            for (s_lo, mk) in masks:
                                m_sb = dm_sb if mk == "d" else em_sb
                                blk = p_sb[:, off + (s_lo - lo):off + (s_lo - lo) + P]
                                nc.vector.tensor_mul(blk, blk, m_sb[:])
                            out.append((p_sb, off, t, lo, hi))
                    return out

                def emit_pv(h, qc, pieces):
                    kvh = h // 2
                    ad = psad.tile([P, 512], f32, tag="ad")
                    # PV: full blocks first (start covers [0:256])
                    ordered = sorted(pieces, key=lambda x: x[3] - x[4])  # wide first
                    n = len(ordered)
                    for i, (p_sb, off, t, lo, hi) in enumerate(ordered):
                        nc.tensor.matmul(
                            ad[:, lo:hi], v_sb[:, t, kvh * D:(kvh + 1) * D],
                            p_sb[:, off:off + (hi - lo)],
                            start=(i == 0), stop=False,
                            skip_group_check=True)
                    # probs sum on DVE
                    ps = pssum.tile([P, SCP], bf16, tag="ps")
                    first = True
                    for (p_sb, off, t, lo, hi) in ordered:
                        src = p_sb[:, off:off + (hi - lo)]
                        if first:
                            nc.vector.tensor_copy(ps[:, lo:hi], src)
                            first = False
                        else:
                            nc.vector.tensor_add(ps[:, lo:hi], ps[:, lo:hi], src)
                    nc.tensor.matmul(ad[:, 256:512], ones_sb[:], ps[:],
                                     start=False, stop=True,
                                     skip_group_check=True)
                    inv = invp.tile([P, SCP], f32, tag="dinv")
                    nc.vector.reciprocal(inv[:], ad[:, 256:512])
                    nc.vector.tensor_mul(
                        attn_sb[:, h, qc * SCP:(qc + 1) * SCP], ad[:, 0:256], inv[:])

                def emit_yproj(qc, mos):
                    o0 = qc * SCP
                    for mg in mos:
                        y_sb = ysbp.tile([P, 4, SCP], bf16, tag="ysb")
                        for mi in range(4):
                            mo = mg * 4 + mi
                            y_ps = psy.tile([P, SCP], f32, tag="y")
                            for h in range(NQC):
                                nc.tensor.matmul(
                                    y_ps[:], wo_sb[:, h, mo * P:(mo + 1) * P],
                                    attn_sb[:, h, o0:o0 + SCP],
                                    start=(h == 0), stop=(h == NQC - 1))
                            cp = nc.vector.tensor_copy if mi % 2 == 0 else nc.scalar.copy
                            cp(y_sb[:, mi, :], y_ps[:])
                        nc.sync.dma_start(
                            out=yT[mg * 4 * P:(mg + 1) * 4 * P, o0:o0 + SCP]
                                .rearrange("(mo p) s -> p mo s", p=P),
                            in_=y_sb[:, :, :])

                # software-pipelined: scores lead PV by two heads; yproj lags
                for qc in range(NCH if PHASES >= 2 else 0):
                    pieces = {}
                    pieces[0] = emit_scores(0, qc)
                    pieces[1] = emit_scores(1, qc)
                    pieces[2] = emit_scores(2, qc)
                    emit_pv(0, qc, pieces[0])
                    pieces[3] = emit_scores(3, qc)
                    emit_pv(1, qc, pieces[1])
                    if PHASES >= 3 and qc >= 1:
                        emit_yproj(qc - 1, range(2))
                    emit_pv(2, qc, pieces[2])
                    if PHASES >= 3 and qc >= 1:
                        emit_yproj(qc - 1, range(2, 4))
                    emit_pv(3, qc, pieces[3])
                if PHASES >= 3:
                    emit_yproj(NCH - 1, range(4))

    nc.compile()
    _CACHE["nc"] = nc
    return nc


def _host_inputs(hidden_states, wq, wk, wv, wo, q_norm_weight, k_norm_weight):
    """Per-core input dicts (8 cores: c = 4*b + g)."""
    import ml_dtypes
    bf16 = ml_dtypes.bfloat16
    f = np.float32
    scale = 1.0 / math.sqrt(D)
    inv_freq = 1.0 / (THETA ** (np.arange(0, D, 2, dtype=np.float64) / D))
    t = np.arange(S, dtype=np.float64)
    freqs = np.outer(t, inv_freq)
    emb = np.concatenate([freqs, freqs], axis=-1)          # [S, D]
    cosT = np.cos(emb).T.astype(f)                         # [D, S]
    sinT = np.sin(emb).T.astype(f)
    qw = (1.0 + q_norm_weight).astype(f)
    kw = (1.0 + k_norm_weight).astype(f)

    R = np.zeros((D, D), f)
    hh = D // 2
    for i in range(hh):
        R[i, i + hh] = -1.0
        R[i + hh, i] = 1.0
    rqT = np.ascontiguousarray((R * qw[None, :]).T).astype(bf16)
    rkT = np.ascontiguousarray((R * kw[None, :]).T).astype(bf16)

    cosq = np.ascontiguousarray(cosT * qw[:, None] * scale).astype(bf16)
    sinq = np.ascontiguousarray(sinT * scale).astype(bf16)
    cosk = np.ascontiguousarray(cosT * kw[:, None]).astype(bf16)
    sink = np.ascontiguousarray(sinT).astype(bf16)
    tabcat_h = np.ascontiguousarray(np.concatenate([cosq, sinq, cosk, sink], axis=0))

    r = np.arange(P)[:, None]
    c = np.arange(P)[None, :]
    dmask = np.where(c >= r, 1.0, 0.0).astype(bf16)        # diag: q_col >= k_row
    emask = np.where(r > c, 1.0, 0.0).astype(bf16)         # edge: k_row > q_col

    hsT = [np.ascontiguousarray(hidden_states[b].T).astype(bf16) for b in range(B)]
    in_maps = []
    for core in range(8):
        b, g = divmod(core, 4)
        in_maps.append({
            "hsT": hsT[b],
            "wqT": np.ascontiguousarray(wq[512 * g:512 * (g + 1), :].T).astype(bf16),
            "wkT": np.ascontiguousarray(wk[256 * g:256 * (g + 1), :].T).astype(bf16),
            "wvT": np.ascontiguousarray(wv[256 * g:256 * (g + 1), :].T).astype(bf16),
            "woT": np.ascontiguousarray(wo[:, 512 * g:512 * (g + 1)].T).astype(bf16),
            "tabcat": tabcat_h,
            "rqT": rqT, "rkT": rkT,
            "onesd": np.ones((P, P), bf16),
            "dmask": dmask, "emask": emask,
        })
    return in_maps


def _postprocess(results):
    out = np.empty((B, S, H), np.float32)
    for b in range(B):
        acc = results[4 * b]["yT"].astype(np.float32).copy()
        for g in range(1, 4):
            acc += results[4 * b + g]["yT"]
        out[b] = acc.T
    return out


def kernel(hidden_states, wq, wk, wv, wo, q_norm_weight, k_norm_weight):
    nc = _build_nc()
    in_maps = _host_inputs(hidden_states, wq, wk, wv, wo, q_norm_weight, k_norm_weight)
    res = run_bass_kernel_spmd(nc, in_maps, list(range(8)))
    return _postprocess(res.results)
